# revision 22
# baseline (speedup 1.0000x reference)
"""Trainium2 Bass kernel for nn_Critic (chain-graph GCN critic).

Math (same folding as validated baseline):
  sa[b,n,:] = 5 node features; chain GCN agg is a tridiagonal stencil; all of
  it up to the ReLU folds into one [16,128] matmul over stacked shifted
  features plus a ones-row that carries conv_b.  LayerNorm + global-add-pool
  reduce to:
     pooled[h] = g[h]*(S1[h] - T) + (B/8)*N*ln_b[h]
  with S1[h] = sum_c r_c x[h,c], T = sum_c r_c mu_c, r = rsqrt(var+eps).
  AllReduce pooled across 8 cores, tiny MLP tail on every core.

Implementation strategy (cost-model driven):
  - x kept in [H=128 partitions, C columns] bf16; main matmul streams
    sat16 [16, C] through the PE with stationary W16.
  - Per-column sums A=sum_h x and B=sum_h x^2 via near-free "ones-column"
    matmuls (out free dim = 1).
  - x transposed chunk-wise to xT via DMA-engine transpose (idle DMA device),
    then S1 accumulated on the PE as 316 rank-128 matvecs (lhsT=r column).
  - PSUM evacuation (relu) split across Act/Pool/DVE engines; squares on DVE.

Sharding: data-parallel over batch, 4 graphs per core, 8 cores.
"""

import numpy as np
import ml_dtypes

import concourse.bass as bass
import concourse.bacc as bacc
import concourse.tile as tile
import concourse.mybir as mybir
import concourse.bass_isa as bass_isa
from concourse import bass_utils

F32 = mybir.dt.float32
BF16 = mybir.dt.bfloat16

N_CORES = 8
B = 32               # global batch
R = B // N_CORES     # graphs per core
NN = 10000           # nodes per graph
H = 128
FIN = 5
K16 = 16
CREAL = R * NN       # real columns per core (40000)
NCHUNK = (CREAL + 127) // 128   # 313 -> padded to 316 for span alignment
EPS = 1e-5

SPAN = 4096          # columns per pipeline span
C = 40448            # padded columns: 316 chunks of 128, 9*4096 + 3584
NCH = C // 128       # 316 chunks
SPANS = [(s * SPAN, min(SPAN, C - s * SPAN)) for s in range((C + SPAN - 1) // SPAN)]
# stats batches: (chunk_lo, chunk_hi), emitted one span after the data is ready
STATS_BATCHES = {4: (0, 128), 8: (128, 256), 9: (256, 316)}

AF = mybir.ActivationFunctionType
OP = mybir.AluOpType

_CACHED = {}


def _build_program(debug_outs=False, use_collective=True, stage=5):
    nc = bacc.Bacc(
        "TRN2", target_bir_lowering=False, debug=False, num_devices=N_CORES
    )

    def inp(name, shape, dt=F32):
        return nc.dram_tensor(name, shape, dt, kind="ExternalInput").ap()

    sat16 = inp("sat16", [K16, C], BF16)
    W16 = inp("W16", [K16, H], BF16)
    g128 = inp("g128", [H])
    bscaled = inp("bscaled", [H])     # (B/8)*NN*ln_b
    w2 = inp("w2", [H, H])
    b2 = inp("b2", [H])
    w3 = inp("w3", [H])
    b3 = inp("b3", [1])
    y_out = nc.dram_tensor("y_out", [1, 1], F32, kind="ExternalOutput").ap()
    dbg = None
    if debug_outs:
        dbg = {
            "dbg_x": nc.dram_tensor("dbg_x", [H, 512], BF16, kind="ExternalOutput").ap(),
            "dbg_xt": nc.dram_tensor("dbg_xt", [H, 512], BF16, kind="ExternalOutput").ap(),
            "dbg_mu": nc.dram_tensor("dbg_mu", [H, NCH], F32, kind="ExternalOutput").ap(),
            "dbg_r": nc.dram_tensor("dbg_r", [H, NCH], F32, kind="ExternalOutput").ap(),
            "dbg_s1": nc.dram_tensor("dbg_s1", [H, 1], F32, kind="ExternalOutput").ap(),
            "dbg_pooled": nc.dram_tensor("dbg_pooled", [H, 1], F32, kind="ExternalOutput").ap(),
        }

    from contextlib import ExitStack

    with tile.TileContext(nc) as tc, ExitStack() as ctx:
        _body(ctx, tc, sat16, W16, g128, bscaled, w2, b2, w3, b3, y_out, dbg,
              use_collective=use_collective, stage=stage)

    nc.compile()
    return nc


def _evac_engine(gi):
    # PSUM readers are Act and DVE only (GPSIMD cannot access PSUM).
    # Alternate within each span (8 groups) so both engines drain PSUM
    # concurrently; 5 act : 3 dve per span matches the DVE side-load.
    return ("act", "dve", "act", "act", "dve", "act", "dve", "act")[gi % 8]


def _body(ctx, tc, sat16, W16, g128, bscaled, w2, b2, w3, b3, y_out, dbg=None,
          use_collective=True, stage=5):
    nc = tc.nc
    consts = ctx.enter_context(tc.tile_pool(name="consts", bufs=1))
    sat_pool = ctx.enter_context(tc.tile_pool(name="sat", bufs=1))
    x_pool = ctx.enter_context(tc.tile_pool(name="x", bufs=2))
    xsq_pool = ctx.enter_context(tc.tile_pool(name="xsq", bufs=2))
    xt_pool = ctx.enter_context(tc.tile_pool(name="xt", bufs=1))
    stats_pool = ctx.enter_context(tc.tile_pool(name="stats", bufs=1))
    mm_psum = ctx.enter_context(tc.tile_pool(name="mmps", bufs=4, space="PSUM"))
    ab_psum = ctx.enter_context(tc.tile_pool(name="abps", bufs=1, space="PSUM"))
    tl_psum = ctx.enter_context(tc.tile_pool(name="tlps", bufs=1, space="PSUM"))
    dram = ctx.enter_context(tc.tile_pool(name="dram", bufs=1, space="DRAM"))

    dma = nc.sync.dma_start

    # ---- constants into SBUF ----
    w16_t = consts.tile([K16, H], BF16)
    dma(out=w16_t[:], in_=W16)
    ones_col = consts.tile([H, 1], BF16)
    nc.vector.memset(ones_col[:], 1.0)
    idn1 = consts.tile([1, 1], F32)
    nc.vector.memset(idn1[:], 1.0)
    eps_t = consts.tile([H, 1], F32)
    nc.vector.memset(eps_t[:], EPS)
    g_t = consts.tile([H, 1], F32)
    dma(out=g_t[:], in_=g128.rearrange("(h o) -> h o", o=1))
    bs_t = consts.tile([H, 1], F32)
    dma(out=bs_t[:], in_=bscaled.rearrange("(h o) -> h o", o=1))
    w2_t = consts.tile([H, H], F32)
    dma(out=w2_t[:], in_=w2)
    b2_t = consts.tile([1, H], F32)
    dma(out=b2_t[:], in_=b2.rearrange("(o h) -> o h", o=1))
    w3_t = consts.tile([1, H], F32)
    dma(out=w3_t[:], in_=w3.rearrange("(o h) -> o h", o=1))
    b3_t = consts.tile([1, 1], F32)
    dma(out=b3_t[:], in_=b3.rearrange("(o h) -> o h", o=1))

    # ---- persistent tiles ----
    xT_all = xt_pool.tile([H, C], BF16)
    A_ps = ab_psum.tile([H, NCH], F32)
    B_ps = ab_psum.tile([H, NCH], F32)
    mu_all = stats_pool.tile([H, NCH], F32)
    r_f32 = stats_pool.tile([H, NCH], F32)
    r_bf = stats_pool.tile([H, NCH], BF16)

    def emit_stats_batch(ta, tb):
        n = tb - ta
        ex2 = stats_pool.tile([H, 128], F32, tag="ex2")
        var = stats_pool.tile([H, 128], F32, tag="var")
        std = stats_pool.tile([H, 128], F32, tag="std")
        scr = stats_pool.tile([H, 128], F32, tag="scr")
        nc.vector.tensor_scalar_mul(out=mu_all[:, ta:tb], in0=A_ps[:, ta:tb],
                                    scalar1=1.0 / H)
        nc.vector.tensor_scalar_mul(out=ex2[:, 0:n], in0=B_ps[:, ta:tb],
                                    scalar1=1.0 / H)
        nc.vector.tensor_mul(out=var[:, 0:n], in0=mu_all[:, ta:tb],
                             in1=mu_all[:, ta:tb])
        nc.vector.tensor_sub(out=var[:, 0:n], in0=ex2[:, 0:n], in1=var[:, 0:n])
        nc.scalar.activation(out=std[:, 0:n], in_=var[:, 0:n], func=AF.Sqrt,
                             bias=eps_t[:, 0:1], scale=1.0)
        nc.vector.reciprocal_approx_accurate(out=r_f32[:, ta:tb],
                                             in_=std[:, 0:n], scratch=scr[:, 0:n])
        nc.vector.tensor_copy(out=r_bf[:, ta:tb], in_=r_f32[:, ta:tb])

    # ---- span pipeline ----
    ab_state = {}

    def emit_ab(sp):
        x_t, xsq_t, off, ln = ab_state[sp]
        t0 = off // 128
        for k in range(ln // 128):
            nc.tensor.matmul(out=A_ps[:, t0 + k:t0 + k + 1],
                             lhsT=x_t[:, k * 128:(k + 1) * 128],
                             rhs=ones_col[:], start=True, stop=True)
            nc.tensor.matmul(out=B_ps[:, t0 + k:t0 + k + 1],
                             lhsT=xsq_t[:, k * 128:(k + 1) * 128],
                             rhs=ones_col[:], start=True, stop=True)

    s1_ps = tl_psum.tile([1, H], F32, tag="s1")

    def emit_s1(ta, tb):
        for t in range(ta, tb):
            nc.tensor.matmul(out=s1_ps[:], lhsT=r_bf[:, t:t + 1],
                             rhs=xT_all[:, t * 128:(t + 1) * 128],
                             start=(t == 0), stop=(t == NCH - 1),
                             skip_group_check=True)

    # prefetch ALL sat spans up-front so main matmuls never queue behind the
    # (much larger) per-span xT transposes on the shared DMA engines
    sat_all = sat_pool.tile([K16, C], BF16)
    for s, (off, ln) in enumerate(SPANS):
        dma(out=sat_all[:, off:off + ln], in_=sat16[:, off:off + ln])

    gi = 0
    for s, (off, ln) in enumerate(SPANS):
        x_t = x_pool.tile([H, SPAN], BF16, tag="x")
        for g in range(ln // 512):
            ps = mm_psum.tile([H, 512], F32, tag="mm")
            nc.tensor.matmul(out=ps[:], lhsT=w16_t[:],
                             rhs=sat_all[:, off + g * 512:off + (g + 1) * 512],
                             start=True, stop=True)
            eng = _evac_engine(gi)
            gi += 1
            xd = x_t[:, g * 512:(g + 1) * 512]
            if eng == "act":
                nc.scalar.activation(out=xd, in_=ps[:], func=AF.Relu, scale=1.0)
            else:
                nc.vector.tensor_scalar_max(out=xd, in0=ps[:], scalar1=0.0)
        # square pass split: Pool takes the HEAD slice (evacuated first, so it
        # starts early and hides its slow rate); DVE (2x mode) takes the tail.
        xsq_t = xsq_pool.tile([H, SPAN], BF16, tag="xsq")
        pv = (ln * 3 // 8) // 128 * 128
        nc.gpsimd.tensor_mul(out=xsq_t[:, 0:pv], in0=x_t[:, 0:pv],
                             in1=x_t[:, 0:pv])
        mid = (pv + (ln - pv) // 2) // 128 * 128
        nc.vector.tensor_mul(out=xsq_t[:, pv:mid], in0=x_t[:, pv:mid],
                             in1=x_t[:, pv:mid])
        nc.vector.tensor_mul(out=xsq_t[:, mid:ln], in0=x_t[:, mid:ln],
                             in1=x_t[:, mid:ln])
        nc.sync.dma_start_transpose(
            out=xT_all[:, off:off + ln].rearrange("p (j h) -> p j h", h=128),
            in_=x_t[:, 0:ln])
        # A/B column-sum matmuls for the PREVIOUS span (one-span lag so the
        # PE never head-of-line blocks on evac/square of the current span).
        if s > 0:
            emit_ab(s - 1)
        ab_state[s] = (x_t, xsq_t, off, ln)
        if s == 8:
            emit_s1(0, 128)
        if s == len(SPANS) - 1:
            emit_s1(128, 256)
            emit_ab(s)
        if s in STATS_BATCHES:
            emit_stats_batch(*STATS_BATCHES[s])
        if s == len(SPANS) - 1:
            emit_s1(256, NCH)

    if stage < 2:
        if dbg is not None:
            dma(out=dbg["dbg_x"], in_=xT_all[:, 0:512])
        dma(out=y_out, in_=b3_t[:])
        return

    # ---- T = sum_c r_c * mu_c ----
    tscr = stats_pool.tile([H, NCH], F32)
    nc.vector.tensor_mul(out=tscr[:], in0=r_f32[:], in1=mu_all[:])
    tred = stats_pool.tile([H, 1], F32)
    nc.vector.tensor_reduce(out=tred[:], in_=tscr[:],
                            axis=mybir.AxisListType.X, op=OP.add)
    T_col = stats_pool.tile([H, 1], F32)
    nc.gpsimd.partition_all_reduce(T_col[:], tred[:], channels=H,
                                   reduce_op=bass_isa.ReduceOp.add)

    # ---- S1 readout (matmul waves were emitted inside the span loop) ----
    s1row = stats_pool.tile([1, H], F32)
    nc.vector.tensor_copy(out=s1row[:], in_=s1_ps[:])
    s1c_ps = tl_psum.tile([H, 1], F32, tag="s1c")
    nc.tensor.transpose(out=s1c_ps[:], in_=s1row[:], identity=idn1[:])
    s1col = stats_pool.tile([H, 1], F32)
    nc.vector.tensor_copy(out=s1col[:], in_=s1c_ps[:])

    if dbg is not None:
        dma(out=dbg["dbg_x"], in_=xT_all[:, 0:512])
        dma(out=dbg["dbg_xt"], in_=xT_all[:, 0:512])
        dma(out=dbg["dbg_mu"], in_=mu_all[:])
        dma(out=dbg["dbg_r"], in_=r_f32[:])
        dma(out=dbg["dbg_s1"], in_=s1col[:])

    # ---- pooled partial = g*(S1 - T) + (B/8)*NN*ln_b ----
    pooled = stats_pool.tile([H, 1], F32)
    nc.vector.tensor_scalar(
        out=pooled[:], in0=s1col[:], scalar1=T_col[:, 0:1], scalar2=None,
        op0=OP.subtract,
    )
    nc.vector.scalar_tensor_tensor(
        out=pooled[:], in0=pooled[:], scalar=g_t[:, 0:1], in1=bs_t[:, 0:1],
        op0=OP.mult, op1=OP.add,
    )
    if dbg is not None:
        dma(out=dbg["dbg_pooled"], in_=pooled[:])
    if stage < 5:
        dma(out=y_out, in_=b3_t[:])
        return

    # ---- AllReduce over the 8 cores ----
    cc_in = dram.tile([H, 1], F32)
    cc_out = dram.tile([H, 1], F32)
    dma(out=cc_in[:], in_=pooled[:])
    if use_collective:
        nc.gpsimd.collective_compute(
            "AllReduce",
            OP.add,
            replica_groups=[list(range(N_CORES))],
            ins=[cc_in[:].opt()],
            outs=[cc_out[:].opt()],
        )
    else:
        dma(out=cc_out[:], in_=cc_in[:])
    pooledf = stats_pool.tile([H, 1], F32)
    dma(out=pooledf[:], in_=cc_out[:])

    # ---- MLP tail (redundant on every core) ----
    t2_ps = tl_psum.tile([1, H], F32, tag="s1")
    nc.tensor.matmul(out=t2_ps[:], lhsT=pooledf[:], rhs=w2_t[:], start=True,
                     stop=True)
    h2 = stats_pool.tile([1, H], F32)
    nc.vector.tensor_add(out=h2[:], in0=t2_ps[:], in1=b2_t[:])
    nc.vector.tensor_scalar_max(out=h2[:], in0=h2[:], scalar1=0.0)
    scrh = stats_pool.tile([1, H], F32)
    yacc = stats_pool.tile([1, 1], F32)
    nc.vector.tensor_mul(out=scrh[:], in0=h2[:], in1=w3_t[:])
    nc.vector.tensor_reduce(
        out=yacc[:], in_=scrh[:], axis=mybir.AxisListType.X, op=OP.add
    )
    yt = stats_pool.tile([1, 1], F32)
    nc.vector.tensor_scalar_add(out=yt[:], in0=yacc[:], scalar1=b3_t[:, 0:1])
    dma(out=y_out, in_=yt[:])


def _host_prep(inputs):
    state = np.asarray(inputs["state"], np.float32)
    action = np.asarray(inputs["action"], np.float32)
    conv_w = np.asarray(inputs["conv_w"], np.float32)
    conv_b = np.asarray(inputs["conv_b"], np.float32)
    ln_g = np.asarray(inputs["ln_g"], np.float32)
    ln_b = np.asarray(inputs["ln_b"], np.float32)
    w2 = np.asarray(inputs["w2"], np.float32)
    b2 = np.asarray(inputs["b2"], np.float32)
    w3 = np.asarray(inputs["w3"], np.float32)
    b3 = np.asarray(inputs["b3"], np.float32)
    src = np.asarray(inputs["src"]).astype(np.int64)
    dst = np.asarray(inputs["dst"]).astype(np.int64)
    bs = int(np.asarray(inputs["bs"]))
    assert bs == B and state.shape == (B, NN * 4) and action.shape == (B, NN)

    # graph preprocessing (host): per-node stencil coefficients
    deg = np.zeros(NN, np.float64)
    np.add.at(deg, dst, 1.0)
    norm = 1.0 / np.sqrt(deg[src] * deg[dst])
    off = src - dst
    assert np.all(np.abs(off) <= 1), "kernel supports chain graphs only"
    cP = np.zeros(NN, np.float64)
    cS = np.zeros(NN, np.float64)
    cN = np.zeros(NN, np.float64)
    np.add.at(cP, dst[off == -1], norm[off == -1])
    np.add.at(cS, dst[off == 0], norm[off == 0])
    np.add.at(cN, dst[off == 1], norm[off == 1])
    CSm = np.stack([cP, cS, cN])
    special = np.where(np.max(np.abs(CSm - 1.0 / 3.0), axis=0) > 1e-6)[0]
    assert set(special.tolist()) <= {0, 1, NN - 2, NN - 1}, special

    sa = np.concatenate([state, action], axis=-1).reshape(B, NN, FIN)
    sa5 = np.ascontiguousarray(sa.transpose(0, 2, 1))  # [B, 5, NN]

    # sat16 [16, B*NN]: rows 0-4 prev, 5-9 self, 10-14 next, 15 ones
    sat = np.zeros((B, K16, NN), np.float32)
    sat[:, 0:5, 1:] = sa5[:, :, :-1]
    sat[:, 5:10, :] = sa5
    sat[:, 10:15, :-1] = sa5[:, :, 1:]
    sat[:, 15, :] = 1.0
    # boundary columns: exact coefficients (x3, since W16 folds the 1/3)
    for n in special:
        if n - 1 >= 0:
            sat[:, 0:5, n] = 3.0 * cP[n] * sa5[:, :, n - 1]
        sat[:, 5:10, n] = 3.0 * cS[n] * sa5[:, :, n]
        if n + 1 < NN:
            sat[:, 10:15, n] = 3.0 * cN[n] * sa5[:, :, n + 1]

    W16 = np.zeros((K16, H), np.float32)
    W16[0:15] = np.concatenate([conv_w, conv_w, conv_w], axis=0) / 3.0
    W16[15] = conv_b

    shared = {
        "W16": W16.astype(ml_dtypes.bfloat16),
        "g128": ln_g,
        "bscaled": (np.float64(R) * NN * ln_b.astype(np.float64)).astype(np.float32),
        "w2": w2,
        "b2": b2,
        "w3": w3.reshape(H),
        "b3": b3.reshape(1),
    }
    in_maps = []
    for c in range(N_CORES):
        m = dict(shared)
        sc = sat[c * R:(c + 1) * R]            # [R, 16, NN]
        sc = sc.transpose(1, 0, 2).reshape(K16, R * NN)
        full = np.zeros((K16, C), np.float32)
        full[:, 0:CREAL] = sc
        m["sat16"] = full.astype(ml_dtypes.bfloat16)
        in_maps.append(m)
    return in_maps


def kernel(**inputs) -> np.ndarray:
    in_maps = _host_prep(inputs)
    if "nc" not in _CACHED:
        _CACHED["nc"] = _build_program()
    res = bass_utils.run_bass_kernel_spmd(
        _CACHED["nc"], in_maps, core_ids=list(range(N_CORES))
    )
    return np.asarray(res.results[0]["y_out"], np.float32)
